# revision 54
# baseline (speedup 1.0000x reference)
"""Trainium2 Bass kernel for the NeuralBloch ODE problem — v10.

Two-pass coarse-grid collocation (no serial carry chain anywhere):

  Coarse grid, stride SB=128: interval k spans fine steps [128k, ...]
  (l_k = 128, last 127).  ubar_k = trapezoid-weighted average of the control
  u over the interval (host-precomputed, like the baseline's host-side
  repack/transpose of u); tbar_k = interval midpoint.  One MLP eval per
  coarse interval:  F_k = W3^T h2(yhat_k, ubar_k, p, tbar_k)  and

      y(tau) = y0 + b3*tau + sum_k c_k(tau) * F_k
      c_k(tau) = clip(tau - t_k0, 0, l_k*h)

  evaluated densely at all 2047 fine points by 16 window matmuls with
  triangular-coefficient stationaries (PSUM rows = fine time points,
  free dim = (comp, batch)), written out as bf16 and upcast on host.
  yhat_k comes from PASS A: the same scheme at stride SA=512 with
  yhat == y0, whose 4 F_A rows give y-estimates at every pass-B
  midpoint through a small prefix-coefficient matmul (split in halves,
  spread to partitions 0:8/32:40 to keep matmul bases legal, so pass
  B starts right after pass-A half 0).  CPU study (exact device-math
  emulation): rel err 1.0184e-2 vs dopri5 (budget 2e-2); the MLP's
  weak y-sensitivity means the constant-y0 predictor in pass A plus
  one Picard refinement saturates the quadrature floor, which itself
  is flat in the stride (the ubar averaging captures the first-order
  control fluctuation exactly).

Layouts: MLP in [feature-partitions x (k,b) free]; the G projection
(3 x cols in PSUM) is staged to SBUF (DVE; gpsimd cannot touch PSUM)
and partition-transposed by SBUF->SBUF DMAs into Gp [k-partitions x
(c,b) free], where dense-output matmuls and single-descriptor-per-
partition output DMAs (out is [T, 3*BC] in DRAM) take over.
Scheduling notes: matmul accumulation groups are atomic to the Tile
scheduler (an open group blocks all other matmuls), so every group
closes immediately; G gathers run every 2 chunks so each dense window
interleaves into the MLP stream ~2 chunks after its data lands; the
dense stage's PSUM->SBUF copies alternate DVE/Act (Copy shares the
tanh activation table, so no table reload).
"""

import numpy as np

B_FULL = 2048
T_FULL = 2048
HID = 128
NCORES = 8
BC = B_FULL // NCORES        # 256
CB = 3 * BC                  # 768
SB = 128                     # pass-B coarse stride (fine steps)
SA = 512                     # pass-A coarse stride
NKB = 16                     # pass-B coarse intervals
NKA = 4                      # pass-A coarse intervals
NW = 16                      # dense output windows (128 fine rows each)
CHUNK = 512                  # MLP column chunk (2 coarse slices * BC)
F32 = np.float32

_CACHE = {}


def _intervals(T, s):
    n = (T - 2) // s + 1
    starts = [i * s for i in range(n)]
    lens = [min((i + 1) * s, T - 1) - i * s for i in range(n)]
    return starts, lens


def _build_nc(nrep=1, cfg=None):
    import concourse.bass as bass
    import concourse.bacc as bacc
    import concourse.mybir as mybir
    from concourse.tile import TileContext

    cfg = dict(cfg or {})
    f32 = mybir.dt.float32
    f32r = mybir.dt.float32r
    bf16 = mybir.dt.bfloat16
    Tanh = mybir.ActivationFunctionType.Tanh
    Copy = mybir.ActivationFunctionType.Copy

    T = T_FULL
    NCOLS = NKB * BC             # 16384
    NCH = NCOLS // CHUNK         # 32 pass-B chunks

    nc = bacc.Bacc(None)
    consts_d = nc.dram_tensor("consts", [128, 264], bf16, kind="ExternalInput")
    bias_d = nc.dram_tensor("biases", [128, 2], f32, kind="ExternalInput")
    xstatB_d = nc.dram_tensor("xstatB", [10, NKB, BC], bf16, kind="ExternalInput")
    xstatA_d = nc.dram_tensor("xstatA", [13, NKA, BC], bf16, kind="ExternalInput")
    y0b3_d = nc.dram_tensor("y0b3", [2, NW * CB], f32r, kind="ExternalInput")
    y0b3A_d = nc.dram_tensor("y0b3A", [2, CB], bf16, kind="ExternalInput")
    mm0st_d = nc.dram_tensor("mm0st", [2, 128], f32r, kind="ExternalInput")
    paG_d = nc.dram_tensor("paG", [34, 40], bf16, kind="ExternalInput")
    paY_d = nc.dram_tensor("paY", [2, 40], bf16, kind="ExternalInput")
    ldA_d = nc.dram_tensor("ldA", [NKB, NW * 128], bf16, kind="ExternalInput")
    out_d = nc.dram_tensor("out", [T, CB], bf16, kind="ExternalOutput")

    with TileContext(nc) as tc:
        with (
            tc.tile_pool(name="const", bufs=1) as cpool,
            tc.tile_pool(name="x", bufs=3) as xpool,
            tc.tile_pool(name="h1", bufs=8) as h1p,
            tc.tile_pool(name="h2", bufs=8) as h2p,
            tc.tile_pool(name="gt", bufs=5) as gtp,
            tc.tile_pool(name="ys", bufs=8) as ysp,
            tc.tile_pool(name="ps1", bufs=2, space="PSUM") as pA1,
            tc.tile_pool(name="ps2", bufs=2, space="PSUM") as pA2,
            tc.tile_pool(name="psg", bufs=2, space="PSUM") as pG,
            tc.tile_pool(name="psy", bufs=2, space="PSUM") as pY,
        ):
            # ---- constants ----
            Cb = cpool.tile([128, 2], f32)
            nc.sync.dma_start(Cb[:, :], bias_d[:, :])
            C = cpool.tile([128, 264], bf16)
            nc.sync.dma_start(C[:, :], consts_d[:, :])
            W2 = C[:, 0:128]
            W1 = C[32:45, 128:256]
            W3 = C[:, 256:259]
            b1 = Cb[:, 0:1]
            b2 = Cb[:, 1:2]

            y0b3 = cpool.tile([2, NW * CB], f32r)
            nc.gpsimd.dma_start(y0b3[:, :], y0b3_d[:, :])
            y0b3A = cpool.tile([2, CB], bf16)
            nc.gpsimd.dma_start(y0b3A[:, :], y0b3A_d[:, :])
            mm0st = cpool.tile([2, 128], f32r)
            nc.gpsimd.dma_start(mm0st[:, :], mm0st_d[:, :])
            paG = cpool.tile([34, 40], bf16)
            nc.gpsimd.dma_start(paG[:, :], paG_d[:, :])
            paY = cpool.tile([2, 40], bf16)
            nc.gpsimd.dma_start(paY[:, :], paY_d[:, :])
            ldA = cpool.tile([NKB, NW * 128], bf16)
            nc.gpsimd.dma_start(ldA[:, :], ldA_d[:, :])

            def mlp_chunk(X, c0, gdst):
                """One 512-col MLP chunk; G row staged into gdst[:, :512]."""
                ps1 = pA1.tile([128, CHUNK], f32, tag="l1")
                nc.tensor.matmul(ps1[:, :], W1, X[32:45, c0:c0 + CHUNK],
                                 True, True)
                h1 = h1p.tile([128, CHUNK], bf16, tag="h1")
                nc.scalar.activation(h1[:, :], ps1[:, :], Tanh, bias=b1)
                ps2 = pA2.tile([128, CHUNK], f32, tag="l2")
                nc.tensor.matmul(ps2[:, :], W2, h1[:, :], True, True)
                h2 = h2p.tile([128, CHUNK], bf16, tag="h2")
                nc.scalar.activation(h2[:, :], ps2[:, :], Tanh, bias=b2)
                psg = pG.tile([3, CHUNK], f32, tag="g")
                nc.tensor.matmul(psg[:, :], W3, h2[:, :], True, True)
                nc.vector.tensor_copy(gdst, psg[:, :])

            def emit_dense(Gp, w, tail=False):
                nrows = 128 if w < NW - 1 else 127
                ys = ysp.tile([128, CB], bf16, tag="ys")
                kr = w + 1
                psys = []
                for hh in range(2):
                    if tail:
                        pool, tg = [(pY, "yw"), (pA1, "l1"),
                                    (pA2, "l2")][(2 * w + hh) % 3]
                    else:
                        pool, tg = pY, "yw"
                    psys.append(pool.tile([128, 384], f32, tag=tg,
                                          name="psy"))
                for hh in range(2):          # shared stationary: mm0 pair
                    hc = 384 * hh
                    nc.tensor.matmul(psys[hh][:, :], mm0st[:, :],
                                     y0b3[:, CB * w + hc:CB * w + hc + 384],
                                     True, False)
                for hh in range(2):          # shared stationary: ldA pair
                    hc = 384 * hh
                    nc.tensor.matmul(psys[hh][:, :],
                                     ldA[0:kr, 128 * w:128 * (w + 1)],
                                     Gp[0:kr, hc:hc + 384], False, True)
                for hh in range(2):
                    hc = 384 * hh
                    if hh == 1:
                        nc.scalar.activation(ys[:, hc:hc + 384],
                                             psys[hh][:, :], Copy)
                    else:
                        nc.vector.tensor_copy(ys[:, hc:hc + 384],
                                              psys[hh][:, :])
                nc.sync.dma_start(out_d[1 + 128 * w: 1 + 128 * w + nrows, :],
                                  ys[0:nrows, :])

            def make_rep_tiles():
                XB = xpool.tile([45, NCOLS], bf16, tag="XB", name="XB")
                XA = xpool.tile([45, NKA * BC], bf16, tag="XA", name="XA")
                gtA = xpool.tile([3, NKA * BC], bf16, tag="gtA", name="gtA")
                Gp = xpool.tile([NKB, CB], bf16, tag="Gp", name="Gp")
                GA = xpool.tile([34, CB], bf16, tag="GA", name="GA")
                yhs = xpool.tile([40, CB], bf16, tag="yhs", name="yhs")
                return dict(XB=XB, XA=XA, gtA=gtA, Gp=Gp, GA=GA, yhs=yhs,
                            pse=[None, None])

            def emit_streams(tl):
                nc.sync.dma_start(
                    tl["XA"][32:45, :].rearrange("p (k b) -> p k b", b=BC),
                    xstatA_d[:, :, :])
                nc.sync.dma_start(
                    tl["XB"][35:45, :].rearrange("p (k b) -> p k b", b=BC),
                    xstatB_d[:, :, :])

            def emit_ga_gather(tl, half):
                r0 = 32 * half
                engsG = [nc.gpsimd, nc.sync, nc.sync]
                for c in range(3):
                    engsG[c].dma_start(
                        tl["GA"][r0:r0 + 2, c * BC:(c + 1) * BC],
                        tl["gtA"][c:c + 1, 512 * half:512 * (half + 1)]
                        .rearrange("p (k b) -> p k b", b=BC))

            def emit_yhat_dma(tl, half):
                k0, p0 = 8 * half, 32 * half
                engsA = [nc.gpsimd, nc.scalar, nc.sync]
                for c in range(3):
                    engsA[c].dma_start(
                        tl["XB"][32 + c:33 + c,
                                 BC * k0:BC * (k0 + 8)].rearrange(
                            "p (k b) -> p k b", b=BC),
                        tl["yhs"][p0:p0 + 8, c * BC:(c + 1) * BC])

            def emit_passA(tl):
                XA, gtA, GA, yhs = (tl["XA"], tl["gtA"], tl["GA"],
                                    tl["yhs"])
                mlp_chunk(XA, 0, gtA[:, 0:CHUNK])
                emit_ga_gather(tl, 0)
                mlp_chunk(XA, CHUNK, gtA[:, CHUNK:2 * CHUNK])
                for hh in range(2):
                    hc = 384 * hh
                    tl["pse"][hh] = pY.tile([40, 384], f32, tag="yw",
                                            name=f"pse{hh}")
                    nc.tensor.matmul(tl["pse"][hh][:, :], paY[:, :],
                                     y0b3A[:, hc:hc + 384], True, False)
                    nc.tensor.matmul(tl["pse"][hh][:, :], paG[0:2, :],
                                     GA[0:2, hc:hc + 384], False, True)
                    nc.vector.tensor_copy(yhs[:, hc:hc + 384],
                                          tl["pse"][hh][:, :])
                emit_yhat_dma(tl, 0)
                emit_ga_gather(tl, 1)

            def emit_pa_h1(tl):
                for hh in range(2):
                    hc = 384 * hh
                    ps1h = pG.tile([40, 384], f32, tag="g",
                                   name=f"pseh{hh}")
                    nc.tensor.matmul(ps1h[32:40, :],
                                     paG[32:34, 32:40],
                                     tl["GA"][32:34, hc:hc + 384],
                                     True, True)
                    nc.vector.tensor_tensor(
                        tl["yhs"][32:40, hc:hc + 384],
                        tl["yhs"][32:40, hc:hc + 384], ps1h[32:40, :],
                        mybir.AluOpType.add)
                emit_yhat_dma(tl, 1)

            def emit_passB(tl):
                XB, Gp = tl["XB"], tl["Gp"]
                engs3 = [nc.gpsimd, nc.sync, nc.sync]
                gt2 = None
                for ci in range(NCH):
                    if ci % 2 == 0:
                        gt2 = gtp.tile([3, 2 * CHUNK], bf16, tag="gt",
                                       name="gt2")
                    c0 = CHUNK * ci
                    mlp_chunk(XB, c0, gt2[:, CHUNK * (ci % 2):
                                          CHUNK * (ci % 2 + 1)])
                    if ci == 0:
                        emit_pa_h1(tl)
                    if ci % 2 == 1:
                        r0 = 2 * ci - 2
                        for c in range(3):
                            engs3[c].dma_start(
                                Gp[r0:r0 + 4, c * BC:(c + 1) * BC],
                                gt2[c:c + 1, :].rearrange(
                                    "p (k b) -> p k b", b=BC))
                    if ci >= 3:
                        emit_dense(Gp, 2 * (ci - 3))
                        emit_dense(Gp, 2 * (ci - 3) + 1)

            def emit_tail(tl):
                for w in range(10, NW):
                    emit_dense(tl["Gp"], w, tail=True)

            # software-pipelined rep loop: rep r-1's dense tail is emitted
            # after rep r's pass A, so its data-gated waits overlap the
            # next rep's MLP on the in-order engine queues.
            prev = None
            for rep in range(nrep):
                tl = make_rep_tiles()
                emit_streams(tl)
                emit_passA(tl)
                if prev is not None:
                    emit_tail(prev)
                emit_passB(tl)
                prev = tl
            emit_tail(prev)
    nc.compile()
    return nc


def _prep_consts(W1, b1v, W2, b2v, W3, b3v):
    import ml_dtypes
    C = np.zeros((128, 264), F32)
    C[:, 0:128] = W2
    C[32:35, 128:256] = W1[0:3]     # y rows
    C[35:39, 128:256] = W1[3:7]     # u rows
    C[39, 128:256] = W1[12]         # t row
    C[40:45, 128:256] = W1[7:12]    # p rows
    C[:, 256:259] = W3
    Cb = np.zeros((128, 2), F32)
    Cb[:, 0] = b1v
    Cb[:, 1] = b2v
    return C.astype(ml_dtypes.bfloat16), Cb


def _ubar(u_c, starts, lens):
    """u_c: (BC, T, 4) -> (4, NK, BC) trapezoid-weighted interval average."""
    NK = len(starts)
    out = np.empty((4, NK, u_c.shape[0]), F32)
    for k, (s0, l) in enumerate(zip(starts, lens)):
        seg = u_c[:, s0:s0 + l + 1, :]
        acc = seg[:, 0, :] + seg[:, -1, :] + 2.0 * seg[:, 1:-1, :].sum(axis=1)
        out[:, k, :] = (acc / (2.0 * l)).T
    return out


def _host_coeffs(t, b3v):
    """Window/prefix coefficient matrices (shared across cores)."""
    import ml_dtypes
    h = float(t[1] - t[0])
    T = T_FULL
    sB, lB = _intervals(T, SB)
    sA, lA = _intervals(T, SA)
    tbarB = np.array([(s0 + l / 2.0) * h for s0, l in zip(sB, lB)], F32)
    tbarA = np.array([(s0 + l / 2.0) * h for s0, l in zip(sA, lA)], F32)

    mm0st = np.zeros((2, 128), F32)
    mm0st[0] = 1.0
    mm0st[1] = (np.arange(128) + 1) * h

    # spread layout: logical yhat rows 0:16 at partitions 0:16, rows
    # 16:32 at partitions 32:48; A-interval rows 0:4 at partitions 0:4,
    # 4:8 at partitions 32:36.
    kmap = np.concatenate([np.arange(8), 8 + np.arange(8)])
    pcol = np.concatenate([np.arange(8), 32 + np.arange(8)])
    paG = np.zeros((34, 40), F32)
    for a in range(NKA):
        r = a if a < 2 else 30 + a
        paG[r, pcol] = np.clip(tbarB[kmap] - sA[a] * h, 0.0, lA[a] * h)
    paG[32:34, 0:32] = 0.0
    paG[0:2, 8:32] = 0.0
    paY = np.zeros((2, 40), F32)
    paY[0, pcol] = 1.0
    paY[1, pcol] = tbarB[kmap]

    def coeff(k, i):
        j = (i - 1) // SB
        if k < j:
            return lB[k] * h
        if k == j:
            return (i - sB[j]) * h
        return 0.0

    ldA = np.zeros((NKB, NW * 128), F32)
    for w in range(NW):
        for m in range(128):
            i = 128 * w + m + 1
            if i >= T:
                continue
            for k in range((i - 1) // SB + 1):
                ldA[k, 128 * w + m] = coeff(k, i)

    bf = ml_dtypes.bfloat16
    return (tbarB, tbarA, sB, lB, sA, lA, h,
            mm0st, paG.astype(bf), paY.astype(bf), ldA.astype(bf))


def _prep_core_inputs(c, y0, t, u, p, consts, coeffs):
    import ml_dtypes
    bf = ml_dtypes.bfloat16
    rows = slice(c * BC, (c + 1) * BC)
    W1_, b1_, W2_, b2_, W3_, b3_ = consts
    (tbarB, tbarA, sB, lB, sA, lA, h,
     mm0st, paGbf, paYbf, ldAbf) = coeffs

    u_c = np.ascontiguousarray(u[rows])          # (BC, T, 4)
    y0_c = y0[rows]                              # (BC, 3)
    p_c = p[rows]                                # (BC, 5)

    xstatB = np.empty((10, NKB, BC), F32)
    xstatB[0:4] = _ubar(u_c, sB, lB)
    xstatB[4] = tbarB[:, None]
    xstatB[5:10] = p_c.T[:, None, :]

    xstatA = np.empty((13, NKA, BC), F32)
    xstatA[0:3] = y0_c.T[:, None, :]
    xstatA[3:7] = _ubar(u_c, sA, lA)
    xstatA[7] = tbarA[:, None]
    xstatA[8:13] = p_c.T[:, None, :]

    y0row = np.ascontiguousarray(y0_c.T).reshape(CB)       # c*BC+b
    b3row = np.repeat(b3_, BC).astype(F32)                 # (CB,)
    y0b3 = np.zeros((2, NW * CB), F32)
    for w in range(NW):
        y0b3[0, CB * w:CB * (w + 1)] = y0row + b3row * (128 * w * h)
        y0b3[1, CB * w:CB * (w + 1)] = b3row
    y0b3A = np.stack([y0row, b3row]).astype(bf)

    Cc, Cb = _prep_consts(W1_, b1_, W2_, b2_, W3_, b3_)
    return {
        "consts": Cc,
        "biases": Cb,
        "xstatB": xstatB.astype(bf),
        "xstatA": xstatA.astype(bf),
        "y0b3": y0b3,
        "y0b3A": y0b3A,
        "mm0st": mm0st,
        "paG": paGbf,
        "paY": paYbf,
        "ldA": ldAbf,
    }


def run(inputs, nrep=1, trace=False, cfg=None):
    from concourse.bass_utils import run_bass_kernel_spmd

    y0 = np.asarray(inputs["y0"], F32)
    t = np.asarray(inputs["t"], F32)
    u = np.asarray(inputs["u"], F32)
    p = np.asarray(inputs["p"], F32)
    consts = tuple(np.asarray(inputs[k], F32)
                   for k in ("W1", "b1", "W2", "b2", "W3", "b3"))

    key = (nrep, str(cfg))
    if key not in _CACHE:
        _CACHE[key] = _build_nc(nrep=nrep, cfg=cfg)
    nc = _CACHE[key]

    coeffs = _host_coeffs(t, consts[5])
    in_maps = [
        _prep_core_inputs(c, y0, t, u, p, consts, coeffs)
        for c in range(NCORES)
    ]
    res = run_bass_kernel_spmd(nc, in_maps, list(range(NCORES)), trace=trace)

    out = np.empty((B_FULL, T_FULL, 3), F32)
    for c in range(NCORES):
        o = np.asarray(res.results[c]["out"], F32).reshape(T_FULL, 3, BC)
        out[c * BC:(c + 1) * BC] = o.transpose(2, 0, 1)
    out[:, 0, :] = y0
    return out, res


def kernel(**inputs):
    out, _ = run(inputs)
    return out
